# revision 22
# baseline (speedup 1.0000x reference)
"""LinearCapsPro forward on 8 TRN2 NeuronCores.

Math: out[b,c] = sqrt(u^T sigma u), u = W_c x_b, sigma = (W_c W_c^T + eps I)^-1.
Host-side fold: G_c = W_c W_c^T + eps I = L_c L_c^T  =>  u^T G^-1 u = ||L_c^-1 u||^2.
With W'_c = L_c^-1 W_c the device kernel is just v = x @ W'^T, then
out[b,c] = sqrt(sum_d v[b, c*16+d]^2) - one big matmul + square + group-sum + sqrt.

Sharding: data-parallel over batch (512 rows/core), W' replicated; no collectives.

Inputs ship as fp8e4 (x scaled by SX, W' by SW; the scale comes back out via the
ACT square's input-scale: (v*1/(SX*SW))^2 == v_true^2). fp8 halves input
staging and HBM traffic AND enables the DoubleRow matmul. Rel err vs the f64
reference is ~1e-2 (tolerance 2e-2), from fp8 quantization of x and W'.

Matmuls run in fp8 DoubleRow perf mode: both operands carry k-plane PAIRS
([128, 2, *] APs, k = k2*256 + j*128 + p), the PE array virtualizes to
128x256 and contracts 256 elements per pass. HW-measured by in-NEFF repeat
slope (wall(repeat=301)-wall(repeat=1))/300: body 23.6us vs 46.4us without
DoubleRow - the full 2x; LDWEIGHTS (256 cols) overlaps the N=400 stream via
the PE's reorder window. (An earlier session rejected DR based on a noisy
differencing measurement - that was wrong.)

Host pre-arranges both operands into the exact SBUF layout so every DMA-in is
a contiguous per-partition copy. x rides the SP HW-DGE ring, w the GPSIMD
SW-DGE ring (parallel descriptor-gen paths; sync+scalar share one HWDGE).

Schedule (per core): stripe(4 x 400 cd-cols) -> k2(8) -> m(4 x 128 rows),
k-outer on early stripes (matches DMA arrival), m-outer on the last stripe so
m0-m2 epilogues overlap the remaining matmuls. Epilogue per (stripe, m):
ACT square (pre-scaled) psum->sbuf, DVE group-sum(16), then per-stripe ACT
sqrt + out-DMA on the SP ring as soon as that stripe's capsules are final.
"""

import sys

import numpy as np

try:
    import concourse  # noqa: F401
except ImportError:  # fresh grading dir: concourse lives in the RL repo
    sys.path.insert(0, "/opt/trn_rl_repo")

B, F, C, D = 4096, 2048, 100, 16
N_CORES = 8
BL = B // N_CORES  # 512 batch rows per core
CD = C * D  # 1600
EPS = 1e-4
KT = F // 128  # 16 contraction tiles of 128
KT2 = KT // 2  # 8 DoubleRow k-pair tiles of 256
MT = BL // 128  # 4 batch tiles per core
NS = 400  # cd-stripe width (4 stripes; 25 capsules each)
ST = CD // NS
SX = 16.0  # fp8 pre-scale for x  (|x| < 6  -> |x*SX| < 96, fp8e4 max 240)
SW = 512.0  # fp8 pre-scale for W' (|W'| < 0.05 -> |W'*SW| < 24)

_cached_nc = None


def build_bass(repeat=1):
    """repeat>1 builds a NEFF with the compute body repeated (same output) -
    used only for in-NEFF slope timing, never for grading."""
    import concourse.bacc as bacc
    import concourse.mybir as mybir
    import concourse.tile as tile

    fp8 = mybir.dt.float8e4
    f32 = mybir.dt.float32
    nc = bacc.Bacc("TRN2", target_bir_lowering=False, debug=False, num_devices=N_CORES)
    xT = nc.dram_tensor("xT", [128, KT2, 2, BL], fp8, kind="ExternalInput")
    wT = nc.dram_tensor("wT", [128, ST, KT2, 2, NS], fp8, kind="ExternalInput")
    out = nc.dram_tensor("out", [BL, C], f32, kind="ExternalOutput")

    with tile.TileContext(nc) as tc:
        with (
            tc.tile_pool(name="xp", bufs=1) as xp,
            tc.tile_pool(name="wp", bufs=1) as wp,
            tc.tile_pool(name="ps", bufs=2, space="PSUM") as psp,
            tc.tile_pool(name="ep", bufs=4) as ep,
            tc.tile_pool(name="rp", bufs=1) as rp,
        ):
            xsb = xp.tile([128, KT2, 2, BL], fp8)
            wsb = wp.tile([128, ST, KT2, 2, NS], fp8)
            for a, b in ((0, 2), (2, 4), (4, 8)):
                nc.sync.dma_start(xsb[:, a:b], xT[:, a:b])
                nc.gpsimd.dma_start(wsb[:, 0, a:b], wT[:, 0, a:b])
            for s in range(1, ST):
                nc.gpsimd.dma_start(wsb[:, s], wT[:, s])
            sq_scale = 1.0 / (SX * SW)
            ncaps = NS // D  # 25
            for r in range(repeat):
                res = [
                    rp.tile([128, C], f32, tag=f"res{m}", name=f"res_r{r}_m{m}")
                    for m in range(MT)
                ]
                for s in range(ST):
                    pss = [
                        psp.tile([128, NS], f32, tag=f"ps{m}", name=f"ps_{s}_{m}_{r}")
                        for m in range(MT)
                    ]
                    order = (
                        [(k, m) for k in range(KT2) for m in range(MT)]
                        if s < ST - 1
                        else [(k, m) for m in range(MT) for k in range(KT2)]
                    )
                    for k, m in order:
                        nc.tensor.matmul(
                            pss[m][:],
                            xsb[:, k, :, m * 128 : (m + 1) * 128],  # lhsT [K, 2, M]
                            wsb[:, s, k, :, :],  # rhs [K, 2, N]
                            start=(k == 0),
                            stop=(k == KT2 - 1),
                            perf_mode=mybir.MatmulPerfMode.DoubleRow,
                        )
                    c0, c1 = s * ncaps, (s + 1) * ncaps
                    for m in range(MT):
                        sq = ep.tile([128, NS], f32, tag="sq", name=f"sq_{s}_{m}_{r}")
                        nc.scalar.activation(
                            sq[:],
                            pss[m][:],
                            mybir.ActivationFunctionType.Square,
                            scale=sq_scale,
                        )
                        nc.vector.reduce_sum(
                            res[m][:, c0:c1],
                            sq[:].rearrange("p (c d) -> p c d", d=D),
                            axis=mybir.AxisListType.X,
                        )
                        nc.scalar.sqrt(res[m][:, c0:c1], res[m][:, c0:c1])
                        nc.sync.dma_start(
                            out[m * 128 : (m + 1) * 128, c0:c1], res[m][:, c0:c1]
                        )
    nc.compile()
    return nc


def prep_inputs(x: np.ndarray, weight: np.ndarray):
    """Host-side fold + fp8 quantize + DoubleRow SBUF-layout pre-arrange."""
    import ml_dtypes

    fp8 = ml_dtypes.float8_e4m3  # IEEE e4m3 (max 240) == TRN FP8_EXP4
    W64 = weight.astype(np.float64)  # [C, D, F]
    G = np.einsum("cdf,cef->cde", W64, W64)
    G[:, np.arange(D), np.arange(D)] += EPS
    L = np.linalg.cholesky(G)
    Wp = np.linalg.solve(L, W64).reshape(CD, F)  # L^-1 W : [CD, F]
    W8 = np.clip(Wp * SW, -240.0, 240.0).astype(fp8)  # [CD, F]
    X8 = np.clip(x.astype(np.float64) * SX, -240.0, 240.0).astype(fp8)  # [B, F]
    # w[p, s, k2, j, n] = W'[s*NS+n, k2*256 + j*128 + p]
    w_sb = np.ascontiguousarray(
        W8.reshape(ST, NS, KT2, 2, 128).transpose(4, 0, 2, 3, 1)
    )
    in_maps = []
    for i in range(N_CORES):
        xi = X8[i * BL : (i + 1) * BL]  # [BL, F]
        # x[p, k2, j, m] = x[m, k2*256 + j*128 + p]
        x_sb = np.ascontiguousarray(xi.reshape(BL, KT2, 2, 128).transpose(3, 1, 2, 0))
        in_maps.append({"xT": x_sb, "wT": w_sb})
    return in_maps


def kernel(x: np.ndarray, weight: np.ndarray) -> np.ndarray:
    global _cached_nc
    x = np.asarray(x)
    weight = np.asarray(weight)
    assert x.shape == (B, F) and weight.shape == (C, D, F), (x.shape, weight.shape)
    in_maps = prep_inputs(x, weight)
    if _cached_nc is None:
        _cached_nc = build_bass()
    from concourse.bass_utils import run_bass_kernel_spmd

    res = run_bass_kernel_spmd(_cached_nc, in_maps, core_ids=list(range(N_CORES)))
    return np.concatenate(
        [res.results[i]["out"] for i in range(N_CORES)], axis=0
    ).astype(np.float32)


# revision 23
# speedup vs baseline: 1.0240x; 1.0240x over previous
"""LinearCapsPro forward on 8 TRN2 NeuronCores.

Math: out[b,c] = sqrt(u^T sigma u), u = W_c x_b, sigma = (W_c W_c^T + eps I)^-1.
Host-side fold: G_c = W_c W_c^T + eps I = L_c L_c^T  =>  u^T G^-1 u = ||L_c^-1 u||^2.
With W'_c = L_c^-1 W_c the device kernel is just v = x @ W'^T, then
out[b,c] = sqrt(sum_d v[b, c*16+d]^2) - one big matmul + square + group-sum + sqrt.

Sharding: data-parallel over batch (512 rows/core), W' replicated; no collectives.

Inputs ship as fp8e4 (x scaled by SX, W' by SW; the scale comes back out via the
ACT square's input-scale: (v*1/(SX*SW))^2 == v_true^2). fp8 halves input
staging and HBM traffic AND enables the DoubleRow matmul. Rel err vs the f64
reference is ~1e-2 (tolerance 2e-2), from fp8 quantization of x and W'.

Matmuls run in fp8 DoubleRow perf mode: both operands carry k-plane PAIRS
([128, 2, *] APs, k = k2*256 + j*128 + p), the PE array virtualizes to
128x256 and contracts 256 elements per pass. HW-measured by in-NEFF repeat
slope (wall(repeat=301)-wall(repeat=1))/300: body 23.6us vs 46.4us without
DoubleRow - the full 2x; LDWEIGHTS (256 cols) overlaps the N=400 stream via
the PE's reorder window. (An earlier session rejected DR based on a noisy
differencing measurement - that was wrong.)

Host pre-arranges both operands into the exact SBUF layout so every DMA-in is
a contiguous per-partition copy. x rides the SP HW-DGE ring, w the GPSIMD
SW-DGE ring (parallel descriptor-gen paths; sync+scalar share one HWDGE).

Schedule (per core): stripe(4 x 400 cd-cols) -> k2(8) -> m(4 x 128 rows),
k-outer on early stripes (matches DMA arrival), m-outer on the last stripe so
m0-m2 epilogues overlap the remaining matmuls. Epilogue per (stripe, m):
ACT square (pre-scaled) psum->sbuf, DVE group-sum(16), then per-stripe ACT
sqrt + out-DMA on the SP ring as soon as that stripe's capsules are final.
"""

import sys

import numpy as np

try:
    import concourse  # noqa: F401
except ImportError:  # fresh grading dir: concourse lives in the RL repo
    sys.path.insert(0, "/opt/trn_rl_repo")

B, F, C, D = 4096, 2048, 100, 16
N_CORES = 8
BL = B // N_CORES  # 512 batch rows per core
CD = C * D  # 1600
EPS = 1e-4
KT = F // 128  # 16 contraction tiles of 128
KT2 = KT // 2  # 8 DoubleRow k-pair tiles of 256
MT = BL // 128  # 4 batch tiles per core
NS = 400  # cd-stripe width (4 stripes; 25 capsules each)
ST = CD // NS
SX = 16.0  # fp8 pre-scale for x  (|x| < 6  -> |x*SX| < 96, fp8e4 max 240)
SW = 512.0  # fp8 pre-scale for W' (|W'| < 0.05 -> |W'*SW| < 24)

_cached_nc = None


def build_bass(repeat=1):
    """repeat>1 builds a NEFF with the compute body repeated (same output) -
    used only for in-NEFF slope timing, never for grading."""
    import concourse.bacc as bacc
    import concourse.mybir as mybir
    import concourse.tile as tile

    fp8 = mybir.dt.float8e4
    f32 = mybir.dt.float32
    nc = bacc.Bacc("TRN2", target_bir_lowering=False, debug=False, num_devices=N_CORES)
    xT = nc.dram_tensor("xT", [128, KT2, 2, BL], fp8, kind="ExternalInput")
    wT = nc.dram_tensor("wT", [128, ST, KT2, 2, NS], fp8, kind="ExternalInput")
    out = nc.dram_tensor("out", [BL, C], f32, kind="ExternalOutput")

    with tile.TileContext(nc) as tc:
        with (
            tc.tile_pool(name="xp", bufs=1) as xp,
            tc.tile_pool(name="wp", bufs=1) as wp,
            tc.tile_pool(name="ps", bufs=2, space="PSUM") as psp,
            tc.tile_pool(name="ep", bufs=4) as ep,
            tc.tile_pool(name="rp", bufs=1) as rp,
        ):
            xsb = xp.tile([128, KT2, 2, BL], fp8)
            wsb = wp.tile([128, ST, KT2, 2, NS], fp8)
            # fine leading k2-pieces: HW-measured 3.5us faster full span than
            # coarse (0,2),(2,4),(4,8) via the loads-in-loop repeat slope -
            # the PE starts on k2=0 earlier and the DR stream is slow enough
            # to absorb the extra descriptor-gens
            for a, b in ((0, 1), (1, 2), (2, 4), (4, 6), (6, 8)):
                nc.sync.dma_start(xsb[:, a:b], xT[:, a:b])
                nc.gpsimd.dma_start(wsb[:, 0, a:b], wT[:, 0, a:b])
            for s in range(1, ST):
                nc.gpsimd.dma_start(wsb[:, s], wT[:, s])
            sq_scale = 1.0 / (SX * SW)
            ncaps = NS // D  # 25
            for r in range(repeat):
                res = [
                    rp.tile([128, C], f32, tag=f"res{m}", name=f"res_r{r}_m{m}")
                    for m in range(MT)
                ]
                for s in range(ST):
                    pss = [
                        psp.tile([128, NS], f32, tag=f"ps{m}", name=f"ps_{s}_{m}_{r}")
                        for m in range(MT)
                    ]
                    order = (
                        [(k, m) for k in range(KT2) for m in range(MT)]
                        if s < ST - 1
                        else [(k, m) for m in range(MT) for k in range(KT2)]
                    )
                    for k, m in order:
                        nc.tensor.matmul(
                            pss[m][:],
                            xsb[:, k, :, m * 128 : (m + 1) * 128],  # lhsT [K, 2, M]
                            wsb[:, s, k, :, :],  # rhs [K, 2, N]
                            start=(k == 0),
                            stop=(k == KT2 - 1),
                            perf_mode=mybir.MatmulPerfMode.DoubleRow,
                        )
                    c0, c1 = s * ncaps, (s + 1) * ncaps
                    for m in range(MT):
                        sq = ep.tile([128, NS], f32, tag="sq", name=f"sq_{s}_{m}_{r}")
                        nc.scalar.activation(
                            sq[:],
                            pss[m][:],
                            mybir.ActivationFunctionType.Square,
                            scale=sq_scale,
                        )
                        nc.vector.reduce_sum(
                            res[m][:, c0:c1],
                            sq[:].rearrange("p (c d) -> p c d", d=D),
                            axis=mybir.AxisListType.X,
                        )
                        nc.scalar.sqrt(res[m][:, c0:c1], res[m][:, c0:c1])
                        nc.sync.dma_start(
                            out[m * 128 : (m + 1) * 128, c0:c1], res[m][:, c0:c1]
                        )
    nc.compile()
    return nc


def prep_inputs(x: np.ndarray, weight: np.ndarray):
    """Host-side fold + fp8 quantize + DoubleRow SBUF-layout pre-arrange."""
    import ml_dtypes

    fp8 = ml_dtypes.float8_e4m3  # IEEE e4m3 (max 240) == TRN FP8_EXP4
    W64 = weight.astype(np.float64)  # [C, D, F]
    G = np.einsum("cdf,cef->cde", W64, W64)
    G[:, np.arange(D), np.arange(D)] += EPS
    L = np.linalg.cholesky(G)
    Wp = np.linalg.solve(L, W64).reshape(CD, F)  # L^-1 W : [CD, F]
    W8 = np.clip(Wp * SW, -240.0, 240.0).astype(fp8)  # [CD, F]
    X8 = np.clip(x.astype(np.float64) * SX, -240.0, 240.0).astype(fp8)  # [B, F]
    # w[p, s, k2, j, n] = W'[s*NS+n, k2*256 + j*128 + p]
    w_sb = np.ascontiguousarray(
        W8.reshape(ST, NS, KT2, 2, 128).transpose(4, 0, 2, 3, 1)
    )
    in_maps = []
    for i in range(N_CORES):
        xi = X8[i * BL : (i + 1) * BL]  # [BL, F]
        # x[p, k2, j, m] = x[m, k2*256 + j*128 + p]
        x_sb = np.ascontiguousarray(xi.reshape(BL, KT2, 2, 128).transpose(3, 1, 2, 0))
        in_maps.append({"xT": x_sb, "wT": w_sb})
    return in_maps


def kernel(x: np.ndarray, weight: np.ndarray) -> np.ndarray:
    global _cached_nc
    x = np.asarray(x)
    weight = np.asarray(weight)
    assert x.shape == (B, F) and weight.shape == (C, D, F), (x.shape, weight.shape)
    in_maps = prep_inputs(x, weight)
    if _cached_nc is None:
        _cached_nc = build_bass()
    from concourse.bass_utils import run_bass_kernel_spmd

    res = run_bass_kernel_spmd(_cached_nc, in_maps, core_ids=list(range(N_CORES)))
    return np.concatenate(
        [res.results[i]["out"] for i in range(N_CORES)], axis=0
    ).astype(np.float32)
